# revision 4
# baseline (speedup 1.0000x reference)
"""Trainium2 Bass kernel for single-head attention with row-major K-reshape.

Reference computation (per batch b):
    Q = x @ W_Q.T ; K = x @ W_K.T ; V = x @ W_V.T          # [S, D]
    K_r = K.reshape(D, S)          # row-major reshape, NOT a transpose
    scores = Q @ K_r / D
    out = softmax(scores, -1) @ V

Shapes: B=4, S=2048, D=1024, f32.

Sharding: 8 cores = (batch b in 0..3) x (pair-rank h in 0..1).  Core (b, h)
computes out[b, h*QB:(h+1)*QB, :].  K_r and V for batch b are computed
cooperatively by the pair (b,0)/(b,1) — each core builds one half and the
halves are exchanged with pair-wise AllGathers (chunked so the transfers
overlap the projection matmuls):

  K_r half:  with S == 2*D the row-major reshape gives
                 K_r[m, g*D + c] = K[2m + g, c]
             so rank g's half is  x[g::2, :] @ W_K.T  — the parity-g rows
             of x ("xp").  Fragment g is exactly global columns
             [g*D, (g+1)*D) of K_r.
  V half:    rank g computes V rows [g*QB, (g+1)*QB) = xq @ W_V.T — the
             same rows as its query block ("xq").

Host-side packing (layout/dtype prep only, numpy, once per call): all
operands are shipped pre-transposed (contraction dim on DRAM rows) so
TensorE does zero transposes: xqT bf16 (V-proj lhsT), xqT8 fp8 (Q-proj
rhs), xpT8 fp8 (K-proj lhsT), wqT8/wkT8 fp8 scaled by 16, wvT bf16.
This also halves HBM load traffic vs f32 (the load half of the kernel
is otherwise DMA-bound at ~358 GB/s).

Precision split (tolerance is 2e-2 relative to max|out|):
  Q-proj, K-proj and the scores matmul run in fp8e4 with
  perf_mode=DoubleRow (2 k-tiles per instruction — true 2x PE throughput,
  measured 216 ns per N=512 DR matmul, same as one bf16 matmul).  The x16
  weight scale keeps W entries (sd 1/32) in fp8e4's normal range; 16*16
  is folded into the softmax exp scale.  Scores only steer the (nearly
  uniform) attention weights, so fp8 noise there is attenuated by
  ~1/sqrt(S) in the output.  The V path (V-proj and attn @ V) stays bf16:
  V errors pass straight to the output.  CPU-simulated rel err of this
  split: 5.6e-3 (bf16 baseline: 4.7e-3; measured on HW: 5.5e-3).

Dataflow per core (TensorE matmul computes out[M,N] = lhsT[K,M].T @ rhs[K,N],
contraction over the partition dim; all operands DMA straight into
[P, NDT, cols] 3D SBUF tiles):
    KRfrag[m, c] = lhsT=xpT8[:, pair, m], rhs=wkT8[:, pair, c]   (fp8 DR)
    Vfrag[s', c] = lhsT=xqT[:, dt, s'],   rhs=wvT[:, dt, c]      (bf16)
    QT8[m, i]    = lhsT=wqT8[:, pair, m], rhs=xqT8[:, pair, i]   (fp8 DR)
    KR/V         = pair AllGathers of the fragments, 2 chunks each
                   (DRAM bounce), pulled into SBUF per chunk
    ST[j, i]     = lhsT=KR8[:, pair, j],  rhs=QT8[:, pair, i]    (fp8 DR)
    ET[j, i]     = exp(ST / (D*256))      (ACT, psum->sbuf bf16)
    rsum[i, 1]   = lhsT=ET[:, i-slice],   rhs=ones  (fused into out loop,
                   reusing the stationary ET tile of the out matmuls)
    O[i, c]      = lhsT=ET[:, i-slice],   rhs=V            (bf16)
    out          = O * (1 / rsum)         (DVE per-partition scalar)
"""

from contextlib import ExitStack

import ml_dtypes
import numpy as np

import concourse.tile as tile
from concourse import bacc, mybir
from concourse.bass_utils import run_bass_kernel_spmd

F32 = mybir.dt.float32
BF16 = mybir.dt.bfloat16
F8 = mybir.dt.float8e4
NP_BF16 = ml_dtypes.bfloat16
NP_F8 = ml_dtypes.float8_e4m3fn
P = 128
WS = 16.0  # fp8 pre-scale for W_Q / W_K


def build_attention(nc, S=2048, D=1024, QB=1024, n_cores=8):
    """Emit the per-core attention program into `nc`. Requires S == 2*D == 2*QB."""
    assert S == 2 * D and QB == D and D % P == 0
    NST = S // P        # seq tiles (16)
    NDT = D // P        # d_model tiles (8)
    NQT = QB // P       # query tiles for this core (8)
    NPR = NDT // 2      # DoubleRow k-tile pairs (4)
    NC = min(512, D)    # matmul free-dim chunk (one PSUM bank of f32)
    NCH_D = D // NC     # chunks over output channels (2)
    NCH_Q = QB // NC    # chunks over queries (2)
    NCHK = 2            # AllGather chunks per gather
    MPC = NDT // NCHK   # fragment tiles per gather chunk (4)
    EXP = mybir.ActivationFunctionType.Exp
    DR = mybir.MatmulPerfMode.DoubleRow
    groups = [[2 * b, 2 * b + 1] for b in range(n_cores // 2)]

    xqt_ap = nc.dram_tensor("xqt", [D, QB], BF16, kind="ExternalInput").ap()
    xqt8_ap = nc.dram_tensor("xqt8", [D, QB], F8, kind="ExternalInput").ap()
    xpt8_ap = nc.dram_tensor("xpt8", [D, D], F8, kind="ExternalInput").ap()
    wqt_ap = nc.dram_tensor("wqt", [D, D], F8, kind="ExternalInput").ap()
    wkt_ap = nc.dram_tensor("wkt", [D, D], F8, kind="ExternalInput").ap()
    wvt_ap = nc.dram_tensor("wvt", [D, D], BF16, kind="ExternalInput").ap()
    out_ap = nc.dram_tensor("out", [QB, D], F32, kind="ExternalOutput").ap()

    with tile.TileContext(nc) as tc, ExitStack() as ctx:
        const_pool = ctx.enter_context(tc.tile_pool(name="const", bufs=1))
        qt_pool = ctx.enter_context(tc.tile_pool(name="qt", bufs=1))
        kr_pool = ctx.enter_context(tc.tile_pool(name="kr", bufs=1))
        v_pool = ctx.enter_context(tc.tile_pool(name="v", bufs=1))
        dram = ctx.enter_context(tc.tile_pool(name="dram", bufs=1, space="DRAM"))
        psum_mm = ctx.enter_context(tc.tile_pool(name="psum_mm", bufs=6, space="PSUM"))

        ones = const_pool.tile([P, 1], BF16)
        nc.vector.memset(ones, 1.0)

        QT8 = qt_pool.tile([P, NDT, QB], F8, name="QT8")
        KR8 = kr_pool.tile([P, NDT, S], F8, name="KR8")
        V = [v_pool.tile([P, D], BF16, tag=f"V{s}", name=f"V{s}") for s in range(NST)]

        # DRAM bounce buffers for the chunked pair AllGathers
        kr_frag_c = [dram.tile([MPC, P, D], F8, name=f"kr_frag{c}") for c in range(NCHK)]
        kr_gath_c = [dram.tile([2, MPC, P, D], F8, name=f"kr_gath{c}") for c in range(NCHK)]
        v_frag_c = [dram.tile([MPC, P, D], BF16, name=f"v_frag{c}") for c in range(NCHK)]
        v_gath_c = [dram.tile([2, MPC, P, D], BF16, name=f"v_gath{c}") for c in range(NCHK)]

        with tc.tile_pool(name="xt", bufs=1) as xt_pool, \
                tc.tile_pool(name="wt", bufs=1) as wt_pool, \
                tc.tile_pool(name="frag", bufs=2) as frag_pool:

            def load3d(src_ap, dst3):
                # pre-transposed operand, DRAM [D, cols] -> SBUF [P, NDT, cols]
                for dt in range(NDT):
                    nc.sync.dma_start(
                        out=dst3[:, dt, :], in_=src_ap[dt * P:(dt + 1) * P, :]
                    )

            xpT8 = xt_pool.tile([P, NDT, D], F8, tag="xpT8", name="xpT8")
            wkT8 = wt_pool.tile([P, NDT, D], F8, tag="wkT8", name="wkT8")
            xqT = xt_pool.tile([P, NDT, QB], BF16, tag="xqT", name="xqT")
            xqT8 = xt_pool.tile([P, NDT, QB], F8, tag="xqT8", name="xqT8")
            wvT = wt_pool.tile([P, NDT, D], BF16, tag="wvT", name="wvT")
            wqT8 = wt_pool.tile([P, NDT, D], F8, tag="wqT8", name="wqT8")

            # K-path operands first (they gate the first matmul + AllGather),
            # the rest prefetch behind them on the same queues.
            load3d(xpt8_ap, xpT8)
            load3d(wkt_ap, wkT8)
            load3d(wvt_ap, wvT)
            load3d(xqt_ap, xqT)
            load3d(xqt8_ap, xqT8)
            load3d(wqt_ap, wqT8)

            # ---- K_r half first: its chunked AllGathers start as soon as
            # ---- half the fragments exist and hide under everything after.
            for mt in range(NDT):
                kf = frag_pool.tile([P, D], F8, tag="kf", name="kf")
                for cch in range(NCH_D):
                    pm = psum_mm.tile([P, NC], F32, tag="pm")
                    for t in range(NPR):
                        nc.tensor.matmul(
                            pm[:],
                            xpT8[:, 2 * t:2 * t + 2, mt * P:(mt + 1) * P],
                            wkT8[:, 2 * t:2 * t + 2, cch * NC:(cch + 1) * NC],
                            start=(t == 0), stop=(t == NPR - 1), perf_mode=DR,
                        )
                    nc.scalar.copy(kf[:, cch * NC:(cch + 1) * NC], pm[:])
                c, j = divmod(mt, MPC)
                nc.scalar.dma_start(out=kr_frag_c[c][j], in_=kf[:])
                if j == MPC - 1:
                    nc.gpsimd.collective_compute(
                        "AllGather", mybir.AluOpType.bypass, replica_groups=groups,
                        ins=[kr_frag_c[c].opt()], outs=[kr_gath_c[c].opt()],
                    )
                    for g in range(2):
                        for jj in range(MPC):
                            nc.scalar.dma_start(
                                out=KR8[:, c * MPC + jj, g * D:(g + 1) * D],
                                in_=kr_gath_c[c][g, jj],
                            )

            # ---- V half: Vfrag[st] = xq @ W_V.T in bf16 ----
            for st in range(NQT):
                vf = frag_pool.tile([P, D], BF16, tag="vf", name="vf")
                for cch in range(NCH_D):
                    pm = psum_mm.tile([P, NC], F32, tag="pm")
                    for dt in range(NDT):
                        nc.tensor.matmul(
                            pm[:],
                            xqT[:, dt, st * P:(st + 1) * P],
                            wvT[:, dt, cch * NC:(cch + 1) * NC],
                            start=(dt == 0), stop=(dt == NDT - 1),
                        )
                    nc.vector.tensor_copy(vf[:, cch * NC:(cch + 1) * NC], pm[:])
                c, j = divmod(st, MPC)
                nc.sync.dma_start(out=v_frag_c[c][j], in_=vf[:])
                if j == MPC - 1:
                    nc.gpsimd.collective_compute(
                        "AllGather", mybir.AluOpType.bypass, replica_groups=groups,
                        ins=[v_frag_c[c].opt()], outs=[v_gath_c[c].opt()],
                    )
                    for g in range(2):
                        for jj in range(MPC):
                            nc.sync.dma_start(
                                out=V[g * NQT + c * MPC + jj][:, :],
                                in_=v_gath_c[c][g, jj],
                            )

            # ---- QT projection (fp8 DoubleRow; KR/V gathers in flight) ----
            for mt in range(NDT):
                for ich in range(NCH_Q):
                    pm = psum_mm.tile([P, NC], F32, tag="pm")
                    for t in range(NPR):
                        nc.tensor.matmul(
                            pm[:],
                            wqT8[:, 2 * t:2 * t + 2, mt * P:(mt + 1) * P],
                            xqT8[:, 2 * t:2 * t + 2, ich * NC:(ich + 1) * NC],
                            start=(t == 0), stop=(t == NPR - 1), perf_mode=DR,
                        )
                    nc.scalar.copy(QT8[:, mt, ich * NC:(ich + 1) * NC], pm[:])

        with tc.tile_pool(name="et", bufs=1) as et_pool, \
                tc.tile_pool(name="ostage", bufs=3) as ostage, \
                tc.tile_pool(name="recip", bufs=1) as recip_pool, \
                tc.tile_pool(name="psum_r", bufs=2, space="PSUM") as psum_r:

            # scores^T and exp: ET[jt][:, ich] = exp(sum KR8.T @ QT8 / (D*WS^2))
            ET = [et_pool.tile([P, QB], BF16, tag=f"ET{j}", name=f"ET{j}") for j in range(NST)]
            for jt in range(NST):
                for ich in range(NCH_Q):
                    pm = psum_mm.tile([P, NC], F32, tag="pm")
                    for t in range(NPR):
                        nc.tensor.matmul(
                            pm[:],
                            KR8[:, 2 * t:2 * t + 2, jt * P:(jt + 1) * P],
                            QT8[:, 2 * t:2 * t + 2, ich * NC:(ich + 1) * NC],
                            start=(t == 0), stop=(t == NPR - 1), perf_mode=DR,
                        )
                    nc.scalar.activation(
                        ET[jt][:, ich * NC:(ich + 1) * NC], pm[:], EXP,
                        scale=1.0 / (D * WS * WS),
                    )

            # out[it][:, cch] = (sum_jt ET.T @ V) / (sum_jt ET.T @ ones).
            # The rowsum matmul shares its stationary ET slice with the two
            # out-chunk matmuls of the same (it, jt), so its weight load is
            # (mostly) free and no separate rowsum phase is needed.
            for it in range(NQT):
                pm0 = psum_mm.tile([P, NC], F32, tag="pm")
                pm1 = psum_mm.tile([P, NC], F32, tag="pm")
                pr = psum_r.tile([P, 1], F32, tag="pr")
                for jt in range(NST):
                    lhsT = ET[jt][:, it * P:(it + 1) * P]
                    nc.tensor.matmul(
                        pm0[:], lhsT, V[jt][:, 0:NC],
                        start=(jt == 0), stop=(jt == NST - 1),
                    )
                    nc.tensor.matmul(
                        pm1[:], lhsT, V[jt][:, NC:2 * NC],
                        start=(jt == 0), stop=(jt == NST - 1),
                    )
                    nc.tensor.matmul(
                        pr[:], lhsT, ones[:],
                        start=(jt == 0), stop=(jt == NST - 1),
                    )
                rc = recip_pool.tile([P, 1], F32, tag="rc", name="rc", bufs=2)
                nc.vector.reciprocal(rc[:], pr[:])
                for cch, pm in ((0, pm0), (1, pm1)):
                    ob = ostage.tile([P, NC], F32, tag="ob", name="ob")
                    nc.vector.tensor_scalar_mul(ob[:], pm[:], rc[:])
                    nc.sync.dma_start(
                        out=out_ap[it * P:(it + 1) * P, cch * NC:(cch + 1) * NC],
                        in_=ob[:],
                    )
    return nc


_CACHE = {}


def _get_nc(S=2048, D=1024, QB=1024):
    key = (S, D, QB)
    if key not in _CACHE:
        nc = bacc.Bacc("TRN2", target_bir_lowering=False, debug=False, num_devices=8)
        build_attention(nc, S=S, D=D, QB=QB, n_cores=8)
        nc.compile()
        _CACHE[key] = nc
    return _CACHE[key]


def _run(x, W_Q, W_K, W_V, **spmd_kwargs):
    B, S, D = x.shape  # (4, 2048, 1024)
    QB = S // 2        # queries per core (1024)
    # host-side operand packing: everything pre-transposed (contraction on
    # DRAM rows); fp8e4 with x16 scale for the Q/K path, bf16 for the V path
    x32 = np.asarray(x, dtype=np.float32)
    wqt = np.ascontiguousarray(np.asarray(W_Q, dtype=np.float32).T * WS).astype(NP_F8)
    wkt = np.ascontiguousarray(np.asarray(W_K, dtype=np.float32).T * WS).astype(NP_F8)
    wvt = np.ascontiguousarray(np.asarray(W_V, dtype=np.float32).T).astype(NP_BF16)
    ws = {"wqt": wqt, "wkt": wkt, "wvt": wvt}
    nc = _get_nc(S=S, D=D, QB=QB)
    in_maps = []
    for core in range(8):
        b, h = core // 2, core % 2
        xqt = np.ascontiguousarray(x32[b, h * QB:(h + 1) * QB, :].T)
        xpt = np.ascontiguousarray(x32[b, h::2, :].T)
        in_maps.append({
            "xqt": xqt.astype(NP_BF16),
            "xqt8": xqt.astype(NP_F8),
            "xpt8": xpt.astype(NP_F8),
            **ws,
        })
    res = run_bass_kernel_spmd(nc, in_maps, list(range(8)), **spmd_kwargs)
    out = np.empty((B, S, D), dtype=np.float32)
    for core in range(8):
        b, h = core // 2, core % 2
        out[b, h * QB:(h + 1) * QB, :] = res.results[core]["out"]
    return out, res


def kernel(x, W_Q, W_K, W_V):
    return _run(x, W_Q, W_K, W_V)[0]


# revision 6
# speedup vs baseline: 1.3754x; 1.3754x over previous
"""Trainium2 Bass kernel for single-head attention with row-major K-reshape.

Reference computation (per batch b):
    Q = x @ W_Q.T ; K = x @ W_K.T ; V = x @ W_V.T          # [S, D]
    K_r = K.reshape(D, S)          # row-major reshape, NOT a transpose
    scores = Q @ K_r / D
    out = softmax(scores, -1) @ V

Shapes: B=4, S=2048, D=1024, f32.

Sharding: 8 cores = (batch b in 0..3) x (pair-rank h in 0..1).  Core (b, h)
computes out[b, h*QB:(h+1)*QB, :].  K_r and V for batch b are computed
cooperatively by the pair (b,0)/(b,1) — each core builds one half and the
halves are exchanged with pair-wise AllGathers (chunked so the transfers
overlap the projection matmuls):

  K_r half:  with S == 2*D the row-major reshape gives
                 K_r[m, g*D + c] = K[2m + g, c]
             so rank g's half is  x[g::2, :] @ W_K.T  — the parity-g rows
             of x ("xp").  Fragment g is exactly global columns
             [g*D, (g+1)*D) of K_r.
  V half:    rank g computes V rows [g*QB, (g+1)*QB) = xq @ W_V.T — the
             same rows as its query block ("xq").

Host-side packing (layout/dtype prep only, numpy, once per call): all
operands are shipped pre-transposed (contraction dim on DRAM rows) so
TensorE does zero transposes: xqT bf16 (V-proj lhsT), xqT8 fp8 (Q-proj
rhs), xpT8 fp8 (K-proj lhsT), wqT8/wkT8 fp8 scaled by 16, wvT bf16.
This also halves HBM load traffic vs f32 (the load half of the kernel
is otherwise DMA-bound at ~358 GB/s).

Precision split (tolerance is 2e-2 relative to max|out|):
  Q-proj, K-proj and the scores matmul run in fp8e4 with
  perf_mode=DoubleRow (2 k-tiles per instruction — true 2x PE throughput,
  measured 216 ns per N=512 DR matmul, same as one bf16 matmul).  The x16
  weight scale keeps W entries (sd 1/32) in fp8e4's normal range; 16*16
  is folded into the softmax exp scale.  Scores only steer the (nearly
  uniform) attention weights, so fp8 noise there is attenuated by
  ~1/sqrt(S) in the output.  The V path (V-proj and attn @ V) stays bf16:
  V errors pass straight to the output.  CPU-simulated rel err of this
  split: 5.6e-3 (bf16 baseline: 4.7e-3; measured on HW: 5.5e-3).

Dataflow per core (TensorE matmul computes out[M,N] = lhsT[K,M].T @ rhs[K,N],
contraction over the partition dim; all operands DMA straight into
[P, NDT, cols] 3D SBUF tiles):
    KRfrag[m, c] = lhsT=xpT8[:, pair, m], rhs=wkT8[:, pair, c]   (fp8 DR)
    Vfrag[s', c] = lhsT=xqT[:, dt, s'],   rhs=wvT[:, dt, c]      (bf16)
    QT8[m, i]    = lhsT=wqT8[:, pair, m], rhs=xqT8[:, pair, i]   (fp8 DR)
    KR/V         = pair AllGathers of the fragments, 2 chunks each
                   (DRAM bounce), pulled into SBUF per chunk
    ST[j, i]     = lhsT=KR8[:, pair, j],  rhs=QT8[:, pair, i]    (fp8 DR)
    ET[j, i]     = exp(ST / (D*256))      (ACT, psum->sbuf bf16)
    rsum[i, 1]   = lhsT=ET[:, i-slice],   rhs=ones  (fused into out loop,
                   reusing the stationary ET tile of the out matmuls)
    O[i, c]      = lhsT=ET[:, i-slice],   rhs=V            (bf16)
    out          = O * (1 / rsum)         (DVE per-partition scalar)
"""

from contextlib import ExitStack

import ml_dtypes
import numpy as np

import concourse.tile as tile
from concourse import bacc, mybir
from concourse.bass_utils import run_bass_kernel_spmd

F32 = mybir.dt.float32
BF16 = mybir.dt.bfloat16
F8 = mybir.dt.float8e4
NP_BF16 = ml_dtypes.bfloat16
NP_F8 = ml_dtypes.float8_e4m3fn
P = 128
WS = 16.0  # fp8 pre-scale for W_Q / W_K


def build_attention(nc, S=2048, D=1024, QB=1024, n_cores=8):
    """Emit the per-core attention program into `nc`. Requires S == 2*D == 2*QB."""
    assert S == 2 * D and QB == D and D % P == 0
    NST = S // P        # seq tiles (16)
    NDT = D // P        # d_model tiles (8)
    NQT = QB // P       # query tiles for this core (8)
    NPR = NDT // 2      # DoubleRow k-tile pairs (4)
    NC = min(512, D)    # matmul free-dim chunk (one PSUM bank of f32)
    NCH_D = D // NC     # chunks over output channels (2)
    NCH_Q = QB // NC    # chunks over queries (2)
    NCHK = 2            # AllGather chunks per gather
    MPC = NDT // NCHK   # fragment tiles per gather chunk (4)
    EXP = mybir.ActivationFunctionType.Exp
    DR = mybir.MatmulPerfMode.DoubleRow
    groups = [[2 * b, 2 * b + 1] for b in range(n_cores // 2)]

    xqt_ap = nc.dram_tensor("xqt", [D, QB], BF16, kind="ExternalInput").ap()
    xqt8_ap = nc.dram_tensor("xqt8", [D, QB], F8, kind="ExternalInput").ap()
    xpt8_ap = nc.dram_tensor("xpt8", [D, D], F8, kind="ExternalInput").ap()
    wqt_ap = nc.dram_tensor("wqt", [D, D], F8, kind="ExternalInput").ap()
    wkt_ap = nc.dram_tensor("wkt", [D, D], F8, kind="ExternalInput").ap()
    wvt_ap = nc.dram_tensor("wvt", [D, D], BF16, kind="ExternalInput").ap()
    out_ap = nc.dram_tensor("out", [QB, D], F32, kind="ExternalOutput").ap()

    with tile.TileContext(nc) as tc, ExitStack() as ctx:
        const_pool = ctx.enter_context(tc.tile_pool(name="const", bufs=1))
        qt_pool = ctx.enter_context(tc.tile_pool(name="qt", bufs=1))
        kr_pool = ctx.enter_context(tc.tile_pool(name="kr", bufs=1))
        v_pool = ctx.enter_context(tc.tile_pool(name="v", bufs=1))
        dram = ctx.enter_context(tc.tile_pool(name="dram", bufs=1, space="DRAM"))
        psum_mm = ctx.enter_context(tc.tile_pool(name="psum_mm", bufs=6, space="PSUM"))

        ones = const_pool.tile([P, 1], BF16)
        nc.vector.memset(ones, 1.0)

        QT8 = qt_pool.tile([P, NDT, QB], F8, name="QT8")
        KR8 = kr_pool.tile([P, NDT, S], F8, name="KR8")
        V = [v_pool.tile([P, D], BF16, tag=f"V{s}", name=f"V{s}") for s in range(NST)]

        # DRAM bounce buffers for the chunked pair AllGathers
        kr_frag_c = [dram.tile([MPC, P, D], F8, name=f"kr_frag{c}") for c in range(NCHK)]
        kr_gath_c = [dram.tile([2, MPC, P, D], F8, name=f"kr_gath{c}") for c in range(NCHK)]
        v_frag_c = [dram.tile([MPC, P, D], BF16, name=f"v_frag{c}") for c in range(NCHK)]
        v_gath_c = [dram.tile([2, MPC, P, D], BF16, name=f"v_gath{c}") for c in range(NCHK)]

        with tc.tile_pool(name="xt", bufs=1) as xt_pool, \
                tc.tile_pool(name="wt", bufs=1) as wt_pool, \
                tc.tile_pool(name="frag", bufs=2) as frag_pool:

            def load3d(src_ap, dst3):
                # pre-transposed operand, DRAM [D, cols] -> SBUF [P, NDT, cols]
                for dt in range(NDT):
                    nc.sync.dma_start(
                        out=dst3[:, dt, :], in_=src_ap[dt * P:(dt + 1) * P, :]
                    )

            xpT8 = xt_pool.tile([P, NDT, D], F8, tag="xpT8", name="xpT8")
            wkT8 = wt_pool.tile([P, NDT, D], F8, tag="wkT8", name="wkT8")
            xqT = xt_pool.tile([P, NDT, QB], BF16, tag="xqT", name="xqT")
            xqT8 = xt_pool.tile([P, NDT, QB], F8, tag="xqT8", name="xqT8")
            wvT = wt_pool.tile([P, NDT, D], BF16, tag="wvT", name="wvT")
            wqT8 = wt_pool.tile([P, NDT, D], F8, tag="wqT8", name="wqT8")

            # K-path operands first (they gate the first matmul + AllGather);
            # later phases' loads are issued at their phase start so the DMA
            # cadence stays spread out.
            load3d(xpt8_ap, xpT8)
            load3d(wkt_ap, wkT8)

            # ---- K_r half first: its chunked AllGathers start as soon as
            # ---- half the fragments exist and hide under everything after.
            for mt in range(NDT):
                kf = frag_pool.tile([P, D], F8, tag="kf", name="kf")
                for cch in range(NCH_D):
                    pm = psum_mm.tile([P, NC], F32, tag="pm")
                    for t in range(NPR):
                        nc.tensor.matmul(
                            pm[:],
                            xpT8[:, 2 * t:2 * t + 2, mt * P:(mt + 1) * P],
                            wkT8[:, 2 * t:2 * t + 2, cch * NC:(cch + 1) * NC],
                            start=(t == 0), stop=(t == NPR - 1), perf_mode=DR,
                        )
                    nc.scalar.copy(kf[:, cch * NC:(cch + 1) * NC], pm[:])
                c, j = divmod(mt, MPC)
                nc.scalar.dma_start(out=kr_frag_c[c][j], in_=kf[:])
                if j == MPC - 1:
                    nc.gpsimd.collective_compute(
                        "AllGather", mybir.AluOpType.bypass, replica_groups=groups,
                        ins=[kr_frag_c[c].opt()], outs=[kr_gath_c[c].opt()],
                    )
                    for g in range(2):
                        for jj in range(MPC):
                            nc.scalar.dma_start(
                                out=KR8[:, c * MPC + jj, g * D:(g + 1) * D],
                                in_=kr_gath_c[c][g, jj],
                            )

            # ---- V half: Vfrag[st] = xq @ W_V.T in bf16 ----
            load3d(xqt_ap, xqT)
            load3d(wvt_ap, wvT)
            load3d(xqt8_ap, xqT8)
            load3d(wqt_ap, wqT8)
            for st in range(NQT):
                vf = frag_pool.tile([P, D], BF16, tag="vf", name="vf")
                for cch in range(NCH_D):
                    pm = psum_mm.tile([P, NC], F32, tag="pm")
                    for dt in range(NDT):
                        nc.tensor.matmul(
                            pm[:],
                            xqT[:, dt, st * P:(st + 1) * P],
                            wvT[:, dt, cch * NC:(cch + 1) * NC],
                            start=(dt == 0), stop=(dt == NDT - 1),
                        )
                    nc.vector.tensor_copy(vf[:, cch * NC:(cch + 1) * NC], pm[:])
                c, j = divmod(st, MPC)
                nc.sync.dma_start(out=v_frag_c[c][j], in_=vf[:])
                if j == MPC - 1:
                    nc.gpsimd.collective_compute(
                        "AllGather", mybir.AluOpType.bypass, replica_groups=groups,
                        ins=[v_frag_c[c].opt()], outs=[v_gath_c[c].opt()],
                    )
                    for g in range(2):
                        for jj in range(MPC):
                            nc.sync.dma_start(
                                out=V[g * NQT + c * MPC + jj][:, :],
                                in_=v_gath_c[c][g, jj],
                            )

            # ---- QT projection (fp8 DoubleRow; KR/V gathers in flight) ----
            for mt in range(NDT):
                for ich in range(NCH_Q):
                    pm = psum_mm.tile([P, NC], F32, tag="pm")
                    for t in range(NPR):
                        nc.tensor.matmul(
                            pm[:],
                            wqT8[:, 2 * t:2 * t + 2, mt * P:(mt + 1) * P],
                            xqT8[:, 2 * t:2 * t + 2, ich * NC:(ich + 1) * NC],
                            start=(t == 0), stop=(t == NPR - 1), perf_mode=DR,
                        )
                    nc.scalar.copy(QT8[:, mt, ich * NC:(ich + 1) * NC], pm[:])

        with tc.tile_pool(name="et", bufs=1) as et_pool, \
                tc.tile_pool(name="ostage", bufs=3) as ostage, \
                tc.tile_pool(name="recip", bufs=1) as recip_pool, \
                tc.tile_pool(name="psum_r", bufs=2, space="PSUM") as psum_r:

            # scores^T and exp: ET[jt][:, ich] = exp(sum KR8.T @ QT8 / (D*WS^2))
            ET = [et_pool.tile([P, QB], BF16, tag=f"ET{j}", name=f"ET{j}") for j in range(NST)]
            for jt in range(NST):
                for ich in range(NCH_Q):
                    pm = psum_mm.tile([P, NC], F32, tag="pm")
                    for t in range(NPR):
                        nc.tensor.matmul(
                            pm[:],
                            KR8[:, 2 * t:2 * t + 2, jt * P:(jt + 1) * P],
                            QT8[:, 2 * t:2 * t + 2, ich * NC:(ich + 1) * NC],
                            start=(t == 0), stop=(t == NPR - 1), perf_mode=DR,
                        )
                    nc.scalar.activation(
                        ET[jt][:, ich * NC:(ich + 1) * NC], pm[:], EXP,
                        scale=1.0 / (D * WS * WS),
                    )

            # out[it][:, cch] = (sum_jt ET.T @ V) / (sum_jt ET.T @ ones).
            # The rowsum matmul shares its stationary ET slice with the two
            # out-chunk matmuls of the same (it, jt), so its weight load is
            # (mostly) free and no separate rowsum phase is needed.
            for it in range(NQT):
                pm0 = psum_mm.tile([P, NC], F32, tag="pm")
                pm1 = psum_mm.tile([P, NC], F32, tag="pm")
                pr = psum_r.tile([P, 1], F32, tag="pr")
                for jt in range(NST):
                    lhsT = ET[jt][:, it * P:(it + 1) * P]
                    nc.tensor.matmul(
                        pm0[:], lhsT, V[jt][:, 0:NC],
                        start=(jt == 0), stop=(jt == NST - 1),
                    )
                    nc.tensor.matmul(
                        pm1[:], lhsT, V[jt][:, NC:2 * NC],
                        start=(jt == 0), stop=(jt == NST - 1),
                    )
                    nc.tensor.matmul(
                        pr[:], lhsT, ones[:],
                        start=(jt == 0), stop=(jt == NST - 1),
                    )
                rc = recip_pool.tile([P, 1], F32, tag="rc", name="rc", bufs=2)
                nc.vector.reciprocal(rc[:], pr[:])
                for cch, pm in ((0, pm0), (1, pm1)):
                    ob = ostage.tile([P, NC], F32, tag="ob", name="ob")
                    nc.vector.tensor_scalar_mul(ob[:], pm[:], rc[:])
                    nc.sync.dma_start(
                        out=out_ap[it * P:(it + 1) * P, cch * NC:(cch + 1) * NC],
                        in_=ob[:],
                    )
    return nc


_CACHE = {}


def _get_nc(S=2048, D=1024, QB=1024):
    key = (S, D, QB)
    if key not in _CACHE:
        nc = bacc.Bacc("TRN2", target_bir_lowering=False, debug=False, num_devices=8)
        build_attention(nc, S=S, D=D, QB=QB, n_cores=8)
        nc.compile()
        _CACHE[key] = nc
    return _CACHE[key]


def _run(x, W_Q, W_K, W_V, **spmd_kwargs):
    B, S, D = x.shape  # (4, 2048, 1024)
    QB = S // 2        # queries per core (1024)
    # host-side operand packing: everything pre-transposed (contraction on
    # DRAM rows); fp8e4 with x16 scale for the Q/K path, bf16 for the V path
    x32 = np.asarray(x, dtype=np.float32)
    wqt = np.ascontiguousarray(np.asarray(W_Q, dtype=np.float32).T * WS).astype(NP_F8)
    wkt = np.ascontiguousarray(np.asarray(W_K, dtype=np.float32).T * WS).astype(NP_F8)
    wvt = np.ascontiguousarray(np.asarray(W_V, dtype=np.float32).T).astype(NP_BF16)
    ws = {"wqt": wqt, "wkt": wkt, "wvt": wvt}
    nc = _get_nc(S=S, D=D, QB=QB)
    in_maps = []
    for core in range(8):
        b, h = core // 2, core % 2
        xqt = np.ascontiguousarray(x32[b, h * QB:(h + 1) * QB, :].T)
        xpt = np.ascontiguousarray(x32[b, h::2, :].T)
        in_maps.append({
            "xqt": xqt.astype(NP_BF16),
            "xqt8": xqt.astype(NP_F8),
            "xpt8": xpt.astype(NP_F8),
            **ws,
        })
    res = run_bass_kernel_spmd(nc, in_maps, list(range(8)), **spmd_kwargs)
    out = np.empty((B, S, D), dtype=np.float32)
    for core in range(8):
        b, h = core // 2, core % 2
        out[b, h * QB:(h + 1) * QB, :] = res.results[core]["out"]
    return out, res


def kernel(x, W_Q, W_K, W_V):
    return _run(x, W_Q, W_K, W_V)[0]


# revision 9
# speedup vs baseline: 1.4557x; 1.0584x over previous
"""Trainium2 Bass kernel for single-head attention with row-major K-reshape.

Reference computation (per batch b):
    Q = x @ W_Q.T ; K = x @ W_K.T ; V = x @ W_V.T          # [S, D]
    K_r = K.reshape(D, S)          # row-major reshape, NOT a transpose
    scores = Q @ K_r / D
    out = softmax(scores, -1) @ V

Shapes: B=4, S=2048, D=1024, f32.

Sharding: 8 cores = (batch b in 0..3) x (pair-rank h in 0..1).  Core (b, h)
computes out[b, h*QB:(h+1)*QB, :].  K_r and V for batch b are computed
cooperatively by the pair (b,0)/(b,1) — each core builds one half and the
halves are exchanged with pair-wise AllGathers (chunked, DRAM bounce):

  K_r half:  with S == 2*D the row-major reshape gives
                 K_r[m, g*D + c] = K[2m + g, c]
             so rank g's half is  x[g::2, :] @ W_K.T  — the parity-g rows
             of x ("xp").  Fragment g is exactly global columns
             [g*D, (g+1)*D) of K_r.
  V half:    rank g computes V rows [g*QB, (g+1)*QB) = xq @ W_V.T — the
             same rows as its query block ("xq").

Host-side packing (layout/dtype prep only, numpy, once per call): all
operands are shipped pre-transposed (contraction dim on DRAM rows) so
TensorE does zero transposes: xqT bf16 (V-proj lhsT), xqT8 fp8 (Q-proj
rhs), xpT8 fp8 (K-proj lhsT), wqT8/wkT8 fp8 scaled by 16, wvT bf16.

Precision plan (tolerance is 2e-2 relative to max|out|; this plan
CPU-simulates to 5.9e-3, measured bf16 baseline was 4.7e-3):
  fp8e4 + perf_mode=DoubleRow (2 k-tiles per instruction, 216 ns per
  N=512 matmul — true 2x) for Q-proj, K-proj, scores AND the attn @ V
  matmul.  W_Q/W_K carry a x16 pre-scale (folded into the exp).  The
  V projection stays bf16 (V errors pass straight to the output; fp8
  there measures 4.5e-2).  The attn @ V matmul survives fp8 via an
  offset trick: with near-uniform attention, e = exp(s) ~= 1, so
      out = (eps @ V8 + colsum_V) / (rowsum(eps) + S),   eps = f8(e - 1)
  where eps carries fp8 noise only on the +-0.15-magnitude deviation
  (attenuated ~1/sqrt(S) in the output) and colsum_V = sum_j V[j,:] is
  computed exactly in bf16 (ones-matmul) and seeded into the PSUM
  accumulation with a K=1 broadcast matmul.

Dataflow per core (TensorE matmul computes out[M,N] = lhsT[K,M].T @ rhs[K,N],
contraction over the partition dim; all operands DMA straight into
[P, NDT, cols] 3D SBUF tiles; inner loops reuse each stationary lhsT
across both output chunks to halve LDWEIGHTS traffic):
    KRfrag[m, c] = lhsT=xpT8[:, pair, m], rhs=wkT8[:, pair, c]   (fp8 DR)
    Vfrag[s', c] = lhsT=xqT[:, dt, s'],   rhs=wvT[:, dt, c]      (bf16)
    QT8[m, i]    = lhsT=wqT8[:, pair, m], rhs=xqT8[:, pair, i]   (fp8 DR)
    KR/V         = pair AllGathers of the fragments, 2 chunks each
                   (DRAM bounce), pulled into SBUF per chunk;
                   V8 = f8(V) cast on DVE per pulled tile
    ST[j, i]     = lhsT=KR8[:, pair, j],  rhs=QT8[:, pair, i]    (fp8 DR)
    et           = exp(ST / (D*256))      (ACT, psum->bf16 ring buffer)
    eps[j, i]    = et - 1 -> fp8          (DVE)
    colsum[c]    = lhsT=ones,  rhs=V[jt]  (bf16, shared stationary)
    O[i, c]      = K=1 seed (ones1, colsum) then lhsT=eps[:, pair, i],
                   rhs=V8[:, pair, c]     (fp8 DR into same PSUM group)
    rsum[i, 1]   = lhsT=eps pair (shared), rhs=ones8 [P,2,1]  (fp8 DR)
    out          = O * (1 / (rsum + S))   (DVE per-partition scalar)
"""

from contextlib import ExitStack

import ml_dtypes
import numpy as np

import concourse.tile as tile
from concourse import bacc, mybir
from concourse.bass_utils import run_bass_kernel_spmd

F32 = mybir.dt.float32
BF16 = mybir.dt.bfloat16
F8 = mybir.dt.float8e4
NP_BF16 = ml_dtypes.bfloat16
NP_F8 = ml_dtypes.float8_e4m3fn
P = 128
WS = 16.0  # fp8 pre-scale for W_Q / W_K


def build_attention(nc, S=2048, D=1024, QB=1024, n_cores=8):
    """Emit the per-core attention program into `nc`. Requires S == 2*D == 2*QB."""
    assert S == 2 * D and QB == D and D % P == 0
    NST = S // P        # seq tiles (16)
    NDT = D // P        # d_model tiles (8)
    NQT = QB // P       # query tiles for this core (8)
    NPR = NDT // 2      # DoubleRow k-tile pairs, d_model contraction (4)
    NPS = NST // 2      # DoubleRow k-tile pairs, seq contraction (8)
    NC = min(512, D)    # matmul free-dim chunk (one PSUM bank of f32)
    NCH_D = D // NC     # chunks over output channels (2)
    NCH_Q = QB // NC    # chunks over queries (2)
    NCHK = 2            # AllGather chunks per gather
    MPC = NDT // NCHK   # fragment tiles per gather chunk (4)
    EXP = mybir.ActivationFunctionType.Exp
    DR = mybir.MatmulPerfMode.DoubleRow
    groups = [[2 * b, 2 * b + 1] for b in range(n_cores // 2)]

    xqt_ap = nc.dram_tensor("xqt", [D, QB], BF16, kind="ExternalInput").ap()
    xqt8_ap = nc.dram_tensor("xqt8", [D, QB], F8, kind="ExternalInput").ap()
    xpt8_ap = nc.dram_tensor("xpt8", [D, D], F8, kind="ExternalInput").ap()
    wqt_ap = nc.dram_tensor("wqt", [D, D], F8, kind="ExternalInput").ap()
    wkt_ap = nc.dram_tensor("wkt", [D, D], F8, kind="ExternalInput").ap()
    wvt_ap = nc.dram_tensor("wvt", [D, D], BF16, kind="ExternalInput").ap()
    out_ap = nc.dram_tensor("out", [QB, D], F32, kind="ExternalOutput").ap()

    with tile.TileContext(nc) as tc, ExitStack() as ctx:
        const_pool = ctx.enter_context(tc.tile_pool(name="const", bufs=1))
        qt_pool = ctx.enter_context(tc.tile_pool(name="qt", bufs=1))
        kr_pool = ctx.enter_context(tc.tile_pool(name="kr", bufs=1))
        v_pool = ctx.enter_context(tc.tile_pool(name="v", bufs=1))
        v8_pool = ctx.enter_context(tc.tile_pool(name="v8", bufs=1))
        eps_pool = ctx.enter_context(tc.tile_pool(name="eps", bufs=1))
        dram = ctx.enter_context(tc.tile_pool(name="dram", bufs=1, space="DRAM"))
        psum_mm = ctx.enter_context(tc.tile_pool(name="psum_mm", bufs=4, space="PSUM"))

        ones = const_pool.tile([P, 1], BF16)        # colsum stationary
        nc.vector.memset(ones, 1.0)
        ones1 = const_pool.tile([1, P], BF16)       # K=1 broadcast stationary
        nc.vector.memset(ones1, 1.0)
        ones8 = const_pool.tile([P, 2, 1], F8)      # DR rowsum rhs
        nc.vector.memset(ones8, 1.0)

        QT8 = qt_pool.tile([P, NDT, QB], F8, name="QT8")
        KR8 = kr_pool.tile([P, NDT, S], F8, name="KR8")
        V = [v_pool.tile([P, D], BF16, tag=f"V{s}", name=f"V{s}") for s in range(NST)]
        V8 = v8_pool.tile([P, NST, D], F8, name="V8")
        EPS = eps_pool.tile([P, NST, QB], F8, name="EPS")

        # DRAM bounce buffers for the chunked pair AllGathers
        kr_frag_c = [dram.tile([MPC, P, D], F8, name=f"kr_frag{c}") for c in range(NCHK)]
        kr_gath_c = [dram.tile([2, MPC, P, D], F8, name=f"kr_gath{c}") for c in range(NCHK)]
        v_frag_c = [dram.tile([MPC, P, D], BF16, name=f"v_frag{c}") for c in range(NCHK)]
        v_gath_c = [dram.tile([2, MPC, P, D], BF16, name=f"v_gath{c}") for c in range(NCHK)]

        with tc.tile_pool(name="xt", bufs=1) as xt_pool, \
                tc.tile_pool(name="wt", bufs=1) as wt_pool, \
                tc.tile_pool(name="frag", bufs=2) as frag_pool:

            def load3d(src_ap, dst3, eng):
                # pre-transposed operand, DRAM [D, cols] -> SBUF [P, NDT, cols]
                for dt in range(NDT):
                    eng.dma_start(out=dst3[:, dt, :], in_=src_ap[dt * P:(dt + 1) * P, :])

            xpT8 = xt_pool.tile([P, NDT, D], F8, tag="xpT8", name="xpT8")
            wkT8 = wt_pool.tile([P, NDT, D], F8, tag="wkT8", name="wkT8")
            xqT = xt_pool.tile([P, NDT, QB], BF16, tag="xqT", name="xqT")
            xqT8 = xt_pool.tile([P, NDT, QB], F8, tag="xqT8", name="xqT8")
            wvT = wt_pool.tile([P, NDT, D], BF16, tag="wvT", name="wvT")
            wqT8 = wt_pool.tile([P, NDT, D], F8, tag="wqT8", name="wqT8")

            # K-path operands gate the first matmul + AllGather: issue them
            # on two different DGE rings so their descriptors go out in
            # parallel; later phases' loads are issued at their phase start.
            load3d(xpt8_ap, xpT8, nc.scalar)
            load3d(wkt_ap, wkT8, nc.sync)

            # ---- K_r half first: its chunked AllGathers start as soon as
            # ---- half the fragments exist and hide under everything after.
            for mt in range(NDT):
                kf = frag_pool.tile([P, D], F8, tag="kf", name="kf")
                pms = [psum_mm.tile([P, NC], F32, tag="pm", name=f"pm{i}") for i in range(NCH_D)]
                for t in range(NPR):
                    for cch in range(NCH_D):
                        nc.tensor.matmul(
                            pms[cch][:],
                            xpT8[:, 2 * t:2 * t + 2, mt * P:(mt + 1) * P],
                            wkT8[:, 2 * t:2 * t + 2, cch * NC:(cch + 1) * NC],
                            start=(t == 0), stop=(t == NPR - 1), perf_mode=DR,
                        )
                for cch in range(NCH_D):
                    nc.scalar.copy(kf[:, cch * NC:(cch + 1) * NC], pms[cch][:])
                c, j = divmod(mt, MPC)
                nc.scalar.dma_start(out=kr_frag_c[c][j], in_=kf[:])
                if j == MPC - 1:
                    nc.gpsimd.collective_compute(
                        "AllGather", mybir.AluOpType.bypass, replica_groups=groups,
                        ins=[kr_frag_c[c].opt()], outs=[kr_gath_c[c].opt()],
                    )
                    for g in range(2):
                        for jj in range(MPC):
                            nc.gpsimd.dma_start(
                                out=KR8[:, c * MPC + jj, g * D:(g + 1) * D],
                                in_=kr_gath_c[c][g, jj],
                            )

            # ---- V half: Vfrag[st] = xq @ W_V.T in bf16 ----
            load3d(xqt_ap, xqT, nc.sync)
            load3d(wvt_ap, wvT, nc.scalar)
            load3d(xqt8_ap, xqT8, nc.sync)
            load3d(wqt_ap, wqT8, nc.scalar)
            for st in range(NQT):
                vf = frag_pool.tile([P, D], BF16, tag="vf", name="vf")
                pms = [psum_mm.tile([P, NC], F32, tag="pm", name=f"pm{i}") for i in range(NCH_D)]
                for dt in range(NDT):
                    for cch in range(NCH_D):
                        nc.tensor.matmul(
                            pms[cch][:],
                            xqT[:, dt, st * P:(st + 1) * P],
                            wvT[:, dt, cch * NC:(cch + 1) * NC],
                            start=(dt == 0), stop=(dt == NDT - 1),
                        )
                for cch in range(NCH_D):
                    nc.vector.tensor_copy(vf[:, cch * NC:(cch + 1) * NC], pms[cch][:])
                c, j = divmod(st, MPC)
                nc.sync.dma_start(out=v_frag_c[c][j], in_=vf[:])
                if j == MPC - 1:
                    nc.gpsimd.collective_compute(
                        "AllGather", mybir.AluOpType.bypass, replica_groups=groups,
                        ins=[v_frag_c[c].opt()], outs=[v_gath_c[c].opt()],
                    )
                    for g in range(2):
                        for jj in range(MPC):
                            jt = g * NQT + c * MPC + jj
                            nc.gpsimd.dma_start(out=V[jt][:, :], in_=v_gath_c[c][g, jj])
                            nc.vector.tensor_copy(V8[:, jt, :], V[jt][:, :])

            # ---- QT projection (fp8 DoubleRow; KR/V gathers in flight) ----
            for mt in range(NDT):
                pms = [psum_mm.tile([P, NC], F32, tag="pm", name=f"pm{i}") for i in range(NCH_Q)]
                for t in range(NPR):
                    for ich in range(NCH_Q):
                        nc.tensor.matmul(
                            pms[ich][:],
                            wqT8[:, 2 * t:2 * t + 2, mt * P:(mt + 1) * P],
                            xqT8[:, 2 * t:2 * t + 2, ich * NC:(ich + 1) * NC],
                            start=(t == 0), stop=(t == NPR - 1), perf_mode=DR,
                        )
                for ich in range(NCH_Q):
                    nc.scalar.copy(QT8[:, mt, ich * NC:(ich + 1) * NC], pms[ich][:])

        with tc.tile_pool(name="ets", bufs=4) as ets_pool, \
                tc.tile_pool(name="csum", bufs=1) as csum_pool, \
                tc.tile_pool(name="ostage", bufs=3) as ostage, \
                tc.tile_pool(name="recip", bufs=1) as recip_pool, \
                tc.tile_pool(name="psum_aux", bufs=2, space="PSUM") as psum_aux:

            # scores^T, exp, eps: EPS[:, jt, i] = exp(ST/(D*WS^2)) - 1 in fp8
            for jt in range(NST):
                pms = [psum_mm.tile([P, NC], F32, tag="pm", name=f"pm{i}") for i in range(NCH_Q)]
                for t in range(NPR):
                    for ich in range(NCH_Q):
                        nc.tensor.matmul(
                            pms[ich][:],
                            KR8[:, 2 * t:2 * t + 2, jt * P:(jt + 1) * P],
                            QT8[:, 2 * t:2 * t + 2, ich * NC:(ich + 1) * NC],
                            start=(t == 0), stop=(t == NPR - 1), perf_mode=DR,
                        )
                for ich in range(NCH_Q):
                    et = ets_pool.tile([P, NC], BF16, tag="et", name="et")
                    nc.scalar.activation(et[:], pms[ich][:], EXP, scale=1.0 / (D * WS * WS))
                    nc.vector.tensor_scalar_add(
                        EPS[:, jt, ich * NC:(ich + 1) * NC], et[:], -1.0
                    )

            # colsum_V[c] = sum_j V[j, c]  (bf16, exact; shared `ones` lhsT)
            colsum = csum_pool.tile([1, D], BF16, name="colsum")
            for cch in range(NCH_D):
                pc = psum_aux.tile([1, NC], F32, tag="pc")
                for jt in range(NST):
                    nc.tensor.matmul(
                        pc[:], ones[:], V[jt][:, cch * NC:(cch + 1) * NC],
                        start=(jt == 0), stop=(jt == NST - 1),
                    )
                nc.scalar.copy(colsum[:, cch * NC:(cch + 1) * NC], pc[:])

            # out[it][:, cch] = (colsum + sum_j eps.T @ V8) / (S + sum_j eps)
            for it in range(NQT):
                pm0 = psum_mm.tile([P, NC], F32, tag="pm")
                pm1 = psum_mm.tile([P, NC], F32, tag="pm")
                pr = psum_aux.tile([P, 1], F32, tag="pr")
                nc.tensor.matmul(pm0[:], ones1[:], colsum[:, 0:NC],
                                 start=True, stop=False, skip_group_check=True)
                nc.tensor.matmul(pm1[:], ones1[:], colsum[:, NC:2 * NC],
                                 start=True, stop=False, skip_group_check=True)
                for t in range(NPS):
                    lhsT = EPS[:, 2 * t:2 * t + 2, it * P:(it + 1) * P]
                    nc.tensor.matmul(pm0[:], lhsT, V8[:, 2 * t:2 * t + 2, 0:NC],
                                     start=False, stop=(t == NPS - 1), perf_mode=DR,
                                     skip_group_check=True)
                    nc.tensor.matmul(pm1[:], lhsT, V8[:, 2 * t:2 * t + 2, NC:2 * NC],
                                     start=False, stop=(t == NPS - 1), perf_mode=DR,
                                     skip_group_check=True)
                    nc.tensor.matmul(pr[:], lhsT, ones8[:],
                                     start=(t == 0), stop=(t == NPS - 1), perf_mode=DR)
                rc = recip_pool.tile([P, 1], F32, tag="rc", name="rc", bufs=2)
                nc.vector.tensor_scalar_add(rc[:], pr[:], float(S))
                nc.vector.reciprocal(rc[:], rc[:])
                for cch, pm in ((0, pm0), (1, pm1)):
                    ob = ostage.tile([P, NC], F32, tag="ob", name="ob")
                    nc.vector.tensor_scalar_mul(ob[:], pm[:], rc[:])
                    nc.sync.dma_start(
                        out=out_ap[it * P:(it + 1) * P, cch * NC:(cch + 1) * NC],
                        in_=ob[:],
                    )
    return nc


_CACHE = {}


def _get_nc(S=2048, D=1024, QB=1024):
    key = (S, D, QB)
    if key not in _CACHE:
        nc = bacc.Bacc("TRN2", target_bir_lowering=False, debug=False, num_devices=8)
        build_attention(nc, S=S, D=D, QB=QB, n_cores=8)
        nc.compile()
        _CACHE[key] = nc
    return _CACHE[key]


def _run(x, W_Q, W_K, W_V, **spmd_kwargs):
    B, S, D = x.shape  # (4, 2048, 1024)
    QB = S // 2        # queries per core (1024)
    # host-side operand packing: everything pre-transposed (contraction on
    # DRAM rows); fp8e4 with x16 scale for the Q/K path, bf16 for the V path
    x32 = np.asarray(x, dtype=np.float32)
    wqt = np.ascontiguousarray(np.asarray(W_Q, dtype=np.float32).T * WS).astype(NP_F8)
    wkt = np.ascontiguousarray(np.asarray(W_K, dtype=np.float32).T * WS).astype(NP_F8)
    wvt = np.ascontiguousarray(np.asarray(W_V, dtype=np.float32).T).astype(NP_BF16)
    ws = {"wqt": wqt, "wkt": wkt, "wvt": wvt}
    nc = _get_nc(S=S, D=D, QB=QB)
    in_maps = []
    for core in range(8):
        b, h = core // 2, core % 2
        xqt = np.ascontiguousarray(x32[b, h * QB:(h + 1) * QB, :].T)
        xpt = np.ascontiguousarray(x32[b, h::2, :].T)
        in_maps.append({
            "xqt": xqt.astype(NP_BF16),
            "xqt8": xqt.astype(NP_F8),
            "xpt8": xpt.astype(NP_F8),
            **ws,
        })
    res = run_bass_kernel_spmd(nc, in_maps, list(range(8)), **spmd_kwargs)
    out = np.empty((B, S, D), dtype=np.float32)
    for core in range(8):
        b, h = core // 2, core % 2
        out[b, h * QB:(h + 1) * QB, :] = res.results[core]["out"]
    return out, res


def kernel(x, W_Q, W_K, W_V):
    return _run(x, W_Q, W_K, W_V)[0]
